# revision 6
# baseline (speedup 1.0000x reference)
"""MoE top-2 gating (CustomNaiveGate_Balance_SMoE) on 8 TRN2 NeuronCores.

Strategy (data parallel over the token dim):
  - inp [32768, 1024] f32 is split into 8 shards of 4096 tokens; each shard is
    host-transposed to xT [1024, 4096] so the contraction dim (d_model) lands on
    SBUF partitions for the PE matmul with zero on-device transposes.
  - The tiny gate weight W [16, 1024] is replicated to every core, pre-arranged
    as wt[p, k*16+e] = W[e, k*128+p] so each 128-d chunk k is a ready matmul rhs.
  - Per core: logits[t, e] computed in PSUM ([128 tokens, 8 groups * 16 experts]
    per super-tile), then a vectorized top-2 (max / is_equal / masked re-max)
    with a reversed-iota trick to get argmax indices with jax's lowest-index
    tie-break.  Scores are softmax over the two kept logits = sigmoid(+-(m1-m2)).
  - Host gathers per-core outputs, rebuilds [32768, 2] indices/scores, and
    computes the load-balance loss from them (exact same math as the reference:
    f_i = selection freq, P_i = mean prob mass per expert; both are plain
    bincounts of the kernel's own outputs).
"""

import numpy as np

import concourse.bass as bass
import concourse.mybir as mybir
from concourse.tile import TileContext
from concourse import bacc
from concourse import bass_utils

P = 128          # SBUF partitions
D = 1024         # d_model
E = 16           # experts
TOPK = 2
N_TOKENS = 32768
N_CORES = 8
NT = N_TOKENS // N_CORES   # tokens per core = 4096
KC = D // P                # contraction chunks = 8
TG = 8                     # token groups (of 128) per super-tile
ST_TOK = TG * P            # tokens per super-tile = 1024
N_ST = NT // ST_TOK        # super-tiles per core = 4
BIG = 1.0e30

F32 = mybir.dt.float32
X = mybir.AxisListType.X
ALU = mybir.AluOpType


def build_nc():
    nc = bacc.Bacc("TRN2", target_bir_lowering=False)

    xT = nc.dram_tensor("xT", [D, NT], F32, kind="ExternalInput")
    wt = nc.dram_tensor("wt", [P, KC * E], F32, kind="ExternalInput")
    riota = nc.dram_tensor("riota", [P, TG * E], F32, kind="ExternalInput")

    r1_o = nc.dram_tensor("r1_o", [P, N_ST * TG], F32, kind="ExternalOutput")
    r2_o = nc.dram_tensor("r2_o", [P, N_ST * TG], F32, kind="ExternalOutput")
    s0_o = nc.dram_tensor("s0_o", [P, N_ST * TG], F32, kind="ExternalOutput")
    s1_o = nc.dram_tensor("s1_o", [P, N_ST * TG], F32, kind="ExternalOutput")

    with TileContext(nc) as tc:
        with (
            tc.tile_pool(name="const", bufs=1) as cpool,
            tc.tile_pool(name="xp", bufs=2) as xpool,
            tc.tile_pool(name="ps", bufs=4, space="PSUM") as ppool,
            tc.tile_pool(name="wk", bufs=3) as wpool,
            tc.tile_pool(name="outp", bufs=3) as opool,
        ):
            wt_s = cpool.tile([P, KC * E], F32)
            nc.sync.dma_start(wt_s[:], wt[:])
            ri_s = cpool.tile([P, TG * E], F32)
            nc.sync.dma_start(ri_s[:], riota[:])
            ri_v = ri_s[:].rearrange("p (g e) -> p g e", e=E)

            # PE warmup matmul reading only wt_s: absorbs the const-DMA wait on
            # the PE clock so every real matmul carries a single sync wait
            # (the fused fp32 Matmult/LDW instruction supports only one).
            dummy_ps = ppool.tile([E, E], F32, tag="dummy")
            nc.tensor.matmul(
                dummy_ps[:], lhsT=wt_s[:, 0:E], rhs=wt_s[:, 0:E],
                start=True, stop=True,
            )

            # [p, k, t] view of xT: row k*128+p, col t
            xT_v = xT[:].rearrange("(k p) t -> p k t", p=P)

            for st in range(N_ST):
                # ---- load xT super-tile with one DMA: [128, 8 chunks, 1024 tok] ----
                xt = xpool.tile([P, KC * ST_TOK], F32, tag="xt")
                nc.sync.dma_start(
                    xt[:].rearrange("p (k t) -> p k t", k=KC),
                    xT_v[:, :, st * ST_TOK:(st + 1) * ST_TOK],
                )

                # ---- logits: psum[t, g*16+e] over 8 contraction chunks ----
                ps = ppool.tile([P, TG * E], F32, tag="ps")
                for g in range(TG):
                    for k in range(KC):
                        nc.tensor.matmul(
                            ps[:, g * E:(g + 1) * E],
                            lhsT=xt[:, k * ST_TOK + g * P: k * ST_TOK + (g + 1) * P],
                            rhs=wt_s[:, k * E:(k + 1) * E],
                            start=(k == 0),
                            stop=(k == KC - 1),
                        )
                lg = ps[:].rearrange("p (g e) -> p g e", e=E)

                # ---- top-1 ----
                m1 = wpool.tile([P, TG], F32, tag="m1")
                nc.vector.reduce_max(m1[:], lg, axis=X)
                m1b = m1[:].unsqueeze(-1).broadcast_to([P, TG, E])
                eq1 = wpool.tile([P, TG * E], F32, tag="eq1")
                nc.vector.tensor_tensor(
                    eq1[:].rearrange("p (g e) -> p g e", e=E), lg, m1b, op=ALU.is_equal
                )
                t1 = wpool.tile([P, TG * E], F32, tag="t1")
                nc.vector.tensor_mul(t1[:], eq1[:], ri_s[:])
                r1 = opool.tile([P, TG], F32, tag="r1")
                nc.vector.reduce_max(
                    r1[:], t1[:].rearrange("p (g e) -> p g e", e=E), axis=X
                )

                # ---- mask out top-1, redo for top-2 ----
                m1sel = wpool.tile([P, TG * E], F32, tag="m1sel")
                nc.vector.tensor_tensor(
                    m1sel[:].rearrange("p (g e) -> p g e", e=E),
                    ri_v,
                    r1[:].unsqueeze(-1).broadcast_to([P, TG, E]),
                    op=ALU.is_equal,
                )
                masked = wpool.tile([P, TG * E], F32, tag="masked")
                nc.vector.scalar_tensor_tensor(
                    masked[:], in0=m1sel[:], scalar=-BIG, in1=ps[:],
                    op0=ALU.mult, op1=ALU.add,
                )
                mkv = masked[:].rearrange("p (g e) -> p g e", e=E)
                m2 = wpool.tile([P, TG], F32, tag="m2")
                nc.vector.reduce_max(m2[:], mkv, axis=X)
                eq2 = wpool.tile([P, TG * E], F32, tag="eq2")
                nc.vector.tensor_tensor(
                    eq2[:].rearrange("p (g e) -> p g e", e=E),
                    mkv,
                    m2[:].unsqueeze(-1).broadcast_to([P, TG, E]),
                    op=ALU.is_equal,
                )
                t2 = wpool.tile([P, TG * E], F32, tag="t2")
                nc.vector.tensor_mul(t2[:], eq2[:], ri_s[:])
                r2 = opool.tile([P, TG], F32, tag="r2")
                nc.vector.reduce_max(
                    r2[:], t2[:].rearrange("p (g e) -> p g e", e=E), axis=X
                )

                # ---- scores: s1 = sigmoid(m2 - m1), s0 = sigmoid(m1 - m2) ----
                dm = wpool.tile([P, TG], F32, tag="dm")
                nc.vector.tensor_sub(dm[:], m2[:], m1[:])
                s1 = opool.tile([P, TG], F32, tag="s1")
                nc.scalar.activation(
                    s1[:], dm[:], mybir.ActivationFunctionType.Sigmoid
                )
                s0 = opool.tile([P, TG], F32, tag="s0")
                nc.scalar.activation(
                    s0[:], dm[:], mybir.ActivationFunctionType.Sigmoid, scale=-1.0
                )

                nc.sync.dma_start(r1_o[:, st * TG:(st + 1) * TG], r1[:])
                nc.sync.dma_start(r2_o[:, st * TG:(st + 1) * TG], r2[:])
                nc.sync.dma_start(s0_o[:, st * TG:(st + 1) * TG], s0[:])
                nc.sync.dma_start(s1_o[:, st * TG:(st + 1) * TG], s1[:])

    # run the bacc passes (sync-wait splitting, register allocation, ...);
    # the pjrt exec path serializes nc as-is and never finalizes it.
    nc.finalize()
    return nc


_NC_CACHE = None


def _get_nc():
    global _NC_CACHE
    if _NC_CACHE is None:
        _NC_CACHE = build_nc()
    return _NC_CACHE


def _host_inputs(inp, W):
    wt = np.ascontiguousarray(
        W.T.reshape(KC, P, E).transpose(1, 0, 2).reshape(P, KC * E)
    ).astype(np.float32)
    ri_row = np.tile((E - np.arange(E)).astype(np.float32), TG)
    riota = np.ascontiguousarray(np.broadcast_to(ri_row, (P, TG * E)))
    in_maps = []
    for c in range(N_CORES):
        shard = np.ascontiguousarray(inp[c * NT:(c + 1) * NT, :].T)
        in_maps.append({"xT": shard, "wt": wt, "riota": riota})
    return in_maps


def _postprocess(results):
    idx_parts, score_parts = [], []
    for res in results:
        r1 = res["r1_o"].reshape(P, N_ST, TG)
        r2 = res["r2_o"].reshape(P, N_ST, TG)
        s0 = res["s0_o"].reshape(P, N_ST, TG)
        s1 = res["s1_o"].reshape(P, N_ST, TG)
        # token t = st*1024 + g*128 + p  ->  order (st, g, p)
        i1 = (E - np.transpose(r1, (1, 2, 0)).reshape(NT)).astype(np.int32)
        i2 = (E - np.transpose(r2, (1, 2, 0)).reshape(NT)).astype(np.int32)
        v0 = np.transpose(s0, (1, 2, 0)).reshape(NT)
        v1 = np.transpose(s1, (1, 2, 0)).reshape(NT)
        idx_parts.append(np.stack([i1, i2], axis=1))
        score_parts.append(np.stack([v0, v1], axis=1))
    top_k_indices = np.concatenate(idx_parts, axis=0)
    top_k_scores = np.concatenate(score_parts, axis=0).astype(np.float32)

    flat_idx = top_k_indices.reshape(-1)
    f_i = np.bincount(flat_idx, minlength=E).astype(np.float64) / N_TOKENS
    P_i = (
        np.bincount(flat_idx, weights=top_k_scores.reshape(-1).astype(np.float64),
                    minlength=E)
        / N_TOKENS
    )
    loss = np.float32(np.sum(f_i * P_i) * E)
    return top_k_indices, top_k_scores, loss


def kernel(inp, W):
    inp = np.asarray(inp, dtype=np.float32)
    W = np.asarray(W, dtype=np.float32)
    nc = _get_nc()
    in_maps = _host_inputs(inp, W)
    out = bass_utils.run_bass_kernel_spmd(
        nc, in_maps, core_ids=list(range(N_CORES))
    )
    return _postprocess(out.results)


# revision 7
# speedup vs baseline: 1.6553x; 1.6553x over previous
"""MoE top-2 gating (CustomNaiveGate_Balance_SMoE) on 8 TRN2 NeuronCores.

Strategy (data parallel over the token dim):
  - inp [32768, 1024] f32 is split into 8 shards of 4096 tokens; each shard is
    host-transposed to xT [1024, 4096] so the contraction dim (d_model) lands on
    SBUF partitions for the PE matmul with zero on-device transposes.
  - The tiny gate weight W [16, 1024] is replicated to every core, padded to 32
    "experts" (rows 16..31 zero) and pre-arranged as wt32[p, k*32+e] =
    W[e, k*128+p] so each 128-d chunk k is a ready matmul lhsT and the full
    [32, 512] PSUM tile is matmul-written.
  - Per core, per 512-token window: 8 accumulating matmuls with the x shard as
    the *moving* operand (fp32 streams 512 tokens/instruction; the tiny wt32 is
    the stationary side) -> logitsT [32 experts, 512 tokens] in PSUM; a DVE
    stream-transpose (32x32 blocks) flips tokens onto partitions; a vectorized
    top-2 (max / is_equal / masked re-max, reversed-iota index trick matching
    jax's lowest-index tie-break) yields indices + sigmoid scores.
  - Host gathers per-core outputs, rebuilds [32768, 2] indices/scores, and
    computes the load-balance loss from them (same math as the reference:
    f_i = selection freq, P_i = mean prob mass; plain bincounts of the
    kernel's own outputs).
"""

import numpy as np

import concourse.bass as bass
import concourse.mybir as mybir
from concourse.tile import TileContext
from concourse import bacc
from concourse import bass_utils

P = 128          # SBUF partitions
D = 1024         # d_model
E = 16           # experts
EP = 32          # padded experts (stream-transpose needs 32-row blocks)
TOPK = 2
N_TOKENS = 32768
N_CORES = 8
NT = N_TOKENS // N_CORES   # tokens per core = 4096
KC = D // P                # contraction chunks = 8
WT = 512                   # tokens per window (one PSUM bank of [32, 512])
NW = NT // WT              # windows per core = 8
NB = WT // EP              # 32-token blocks per window = 16
BIG = 1.0e30

F32 = mybir.dt.float32
X = mybir.AxisListType.X
ALU = mybir.AluOpType


def build_nc():
    nc = bacc.Bacc("TRN2", target_bir_lowering=False)

    xT = nc.dram_tensor("xT", [D, NT], F32, kind="ExternalInput")
    wt = nc.dram_tensor("wt", [P, KC * EP], F32, kind="ExternalInput")
    riota = nc.dram_tensor("riota", [EP, E], F32, kind="ExternalInput")

    # per-window outputs land in [:, w*NB:(w+1)*NB]
    r1_o = nc.dram_tensor("r1_o", [EP, NW * NB], F32, kind="ExternalOutput")
    r2_o = nc.dram_tensor("r2_o", [EP, NW * NB], F32, kind="ExternalOutput")
    s0_o = nc.dram_tensor("s0_o", [EP, NW * NB], F32, kind="ExternalOutput")
    s1_o = nc.dram_tensor("s1_o", [EP, NW * NB], F32, kind="ExternalOutput")

    with TileContext(nc) as tc:
        with (
            tc.tile_pool(name="const", bufs=1) as cpool,
            tc.tile_pool(name="xp", bufs=3) as xpool,
            tc.tile_pool(name="ps", bufs=4, space="PSUM") as ppool,
            tc.tile_pool(name="wk", bufs=3) as wpool,
            tc.tile_pool(name="outp", bufs=3) as opool,
        ):
            wt_s = cpool.tile([P, KC * EP], F32)
            nc.sync.dma_start(wt_s[:], wt[:])
            ri_s = cpool.tile([EP, E], F32)
            nc.sync.dma_start(ri_s[:], riota[:])
            # [32 tok, NB blocks, 16 experts] broadcast views of the consts
            ri_b = ri_s[:].unsqueeze(1).broadcast_to([EP, NB, E])

            # PE warmup matmul reading only wt_s: absorbs the const-DMA wait on
            # the PE clock so real matmuls keep a single sync wait each.
            dummy_ps = ppool.tile([EP, EP], F32, tag="dummy")
            nc.tensor.matmul(
                dummy_ps[:], lhsT=wt_s[:, 0:EP], rhs=wt_s[:, 0:EP],
                start=True, stop=True,
            )

            # [p, k, t] view of xT: row k*128+p, col t
            xT_v = xT[:].rearrange("(k p) t -> p k t", p=P)

            for w in range(NW):
                # ---- load window: [128 d, 8 chunks, 512 tokens], one DMA ----
                xt = xpool.tile([P, KC * WT], F32, tag="xt")
                nc.sync.dma_start(
                    xt[:].rearrange("p (k t) -> p k t", k=KC),
                    xT_v[:, :, w * WT:(w + 1) * WT],
                )

                # ---- logitsT [32 experts, 512 tokens] in PSUM ----
                ps = ppool.tile([EP, WT], F32, tag="ps")
                for k in range(KC):
                    nc.tensor.matmul(
                        ps[:],
                        lhsT=wt_s[:, k * EP:(k + 1) * EP],
                        rhs=xt[:, k * WT:(k + 1) * WT],
                        start=(k == 0),
                        stop=(k == KC - 1),
                    )

                # ---- stream-transpose 32x32 blocks: [32e, 512t] -> tokens on
                # partitions: tr[p, b*32+e] = logits(tok w*512+b*32+p, e) ----
                tr = wpool.tile([EP, WT], F32, tag="tr")
                nc.vector.transpose(tr[:], ps[:])
                lg = tr[:].rearrange("p (b e) -> p b e", e=EP)[:, :, 0:E]

                # ---- top-1 ----
                m1 = wpool.tile([EP, NB], F32, tag="m1")
                nc.vector.reduce_max(m1[:], lg, axis=X)
                eq1 = wpool.tile([EP, NB * E], F32, tag="eq1")
                eq1v = eq1[:].rearrange("p (b e) -> p b e", e=E)
                nc.vector.tensor_tensor(
                    eq1v, lg, m1[:].unsqueeze(-1).broadcast_to([EP, NB, E]),
                    op=ALU.is_equal,
                )
                t1 = wpool.tile([EP, NB * E], F32, tag="t1")
                t1v = t1[:].rearrange("p (b e) -> p b e", e=E)
                nc.vector.tensor_tensor(t1v, eq1v, ri_b, op=ALU.mult)
                r1 = opool.tile([EP, NB], F32, tag="r1")
                nc.vector.reduce_max(r1[:], t1v, axis=X)

                # ---- mask top-1 out, redo for top-2 ----
                m1sel = wpool.tile([EP, NB * E], F32, tag="m1sel")
                m1selv = m1sel[:].rearrange("p (b e) -> p b e", e=E)
                nc.vector.tensor_tensor(
                    m1selv, ri_b, r1[:].unsqueeze(-1).broadcast_to([EP, NB, E]),
                    op=ALU.is_equal,
                )
                masked = wpool.tile([EP, NB * E], F32, tag="masked")
                maskedv = masked[:].rearrange("p (b e) -> p b e", e=E)
                nc.vector.scalar_tensor_tensor(
                    maskedv, in0=m1selv, scalar=-BIG, in1=lg,
                    op0=ALU.mult, op1=ALU.add,
                )
                m2 = wpool.tile([EP, NB], F32, tag="m2")
                nc.vector.reduce_max(m2[:], maskedv, axis=X)
                eq2 = wpool.tile([EP, NB * E], F32, tag="eq2")
                eq2v = eq2[:].rearrange("p (b e) -> p b e", e=E)
                nc.vector.tensor_tensor(
                    eq2v, maskedv, m2[:].unsqueeze(-1).broadcast_to([EP, NB, E]),
                    op=ALU.is_equal,
                )
                t2 = wpool.tile([EP, NB * E], F32, tag="t2")
                t2v = t2[:].rearrange("p (b e) -> p b e", e=E)
                nc.vector.tensor_tensor(t2v, eq2v, ri_b, op=ALU.mult)
                r2 = opool.tile([EP, NB], F32, tag="r2")
                nc.vector.reduce_max(r2[:], t2v, axis=X)

                # ---- scores: s1 = sigmoid(m2 - m1), s0 = sigmoid(m1 - m2) ----
                dm = wpool.tile([EP, NB], F32, tag="dm")
                nc.vector.tensor_sub(dm[:], m2[:], m1[:])
                s1 = opool.tile([EP, NB], F32, tag="s1")
                nc.scalar.activation(
                    s1[:], dm[:], mybir.ActivationFunctionType.Sigmoid
                )
                s0 = opool.tile([EP, NB], F32, tag="s0")
                nc.scalar.activation(
                    s0[:], dm[:], mybir.ActivationFunctionType.Sigmoid, scale=-1.0
                )

                nc.sync.dma_start(r1_o[:, w * NB:(w + 1) * NB], r1[:])
                nc.sync.dma_start(r2_o[:, w * NB:(w + 1) * NB], r2[:])
                nc.sync.dma_start(s0_o[:, w * NB:(w + 1) * NB], s0[:])
                nc.sync.dma_start(s1_o[:, w * NB:(w + 1) * NB], s1[:])

    # run the bacc passes (sync-wait splitting, register allocation, ...);
    # the pjrt exec path serializes nc as-is and never finalizes it.
    nc.finalize()
    return nc


_NC_CACHE = None


def _get_nc():
    global _NC_CACHE
    if _NC_CACHE is None:
        _NC_CACHE = build_nc()
    return _NC_CACHE


def _host_inputs(inp, W):
    # wt32[p, k*32+e] = W[e, k*128+p] for e<16, zero-padded to 32 "experts"
    Wp = np.zeros((EP, D), np.float32)
    Wp[:E] = W
    wt = np.ascontiguousarray(
        Wp.T.reshape(KC, P, EP).transpose(1, 0, 2).reshape(P, KC * EP)
    )
    riota = np.ascontiguousarray(
        np.broadcast_to((E - np.arange(E)).astype(np.float32), (EP, E))
    )
    in_maps = []
    for c in range(N_CORES):
        shard = np.ascontiguousarray(inp[c * NT:(c + 1) * NT, :].T)
        in_maps.append({"xT": shard, "wt": wt, "riota": riota})
    return in_maps


def _postprocess(results):
    idx_parts, score_parts = [], []
    for res in results:
        r1 = res["r1_o"].reshape(EP, NW, NB)
        r2 = res["r2_o"].reshape(EP, NW, NB)
        s0 = res["s0_o"].reshape(EP, NW, NB)
        s1 = res["s1_o"].reshape(EP, NW, NB)
        # token t = w*512 + b*32 + p  ->  order (w, b, p)
        i1 = (E - np.transpose(r1, (1, 2, 0)).reshape(NT)).astype(np.int32)
        i2 = (E - np.transpose(r2, (1, 2, 0)).reshape(NT)).astype(np.int32)
        v0 = np.transpose(s0, (1, 2, 0)).reshape(NT)
        v1 = np.transpose(s1, (1, 2, 0)).reshape(NT)
        idx_parts.append(np.stack([i1, i2], axis=1))
        score_parts.append(np.stack([v0, v1], axis=1))
    top_k_indices = np.concatenate(idx_parts, axis=0)
    top_k_scores = np.concatenate(score_parts, axis=0).astype(np.float32)

    flat_idx = top_k_indices.reshape(-1)
    f_i = np.bincount(flat_idx, minlength=E).astype(np.float64) / N_TOKENS
    P_i = (
        np.bincount(flat_idx, weights=top_k_scores.reshape(-1).astype(np.float64),
                    minlength=E)
        / N_TOKENS
    )
    loss = np.float32(np.sum(f_i * P_i) * E)
    return top_k_indices, top_k_scores, loss


def kernel(inp, W):
    inp = np.asarray(inp, dtype=np.float32)
    W = np.asarray(W, dtype=np.float32)
    nc = _get_nc()
    in_maps = _host_inputs(inp, W)
    out = bass_utils.run_bass_kernel_spmd(
        nc, in_maps, core_ids=list(range(N_CORES))
    )
    return _postprocess(out.results)
